# revision 28
# baseline (speedup 1.0000x reference)
"""FM bi-interaction (embedding_lookup) Trainium2 kernel.

out[n, k] = 0.5 * ((x @ E)^2 - (x*x) @ (E*E))[n, k] * mask[n]
mask[n] = 1 if n in train_idx else 0

Strategy (data-parallel over rows, 8 NeuronCores):
- Only rows present in train_idx have nonzero output (~11k of 20k). The host
  gathers the unique train rows, splits them evenly across the 8 cores, and
  scatters the per-row results back into a zero output — no on-device mask.
- x is uploaded in bf16 (the 2e-2 rel-err gate leaves ~40x headroom), halving
  HBM traffic; E is pre-scaled by 1/sqrt(2) on the host so the 0.5 factor
  folds into the matmuls (out = L^2 - R with L = x@E', R = x^2@E'^2).
- Host packs x into the exact SBUF tile layout ([128 f-partitions, 16
  f-tiles, w rows] per block, f padded 10000->10240 — tiles must span all 128
  partitions or DMA throughput collapses), so every x DMA is one ~1.9 MB
  transfer with fully contiguous per-partition lines. DMAs alternate between
  the SP and ACT HWDGE rings to overlap.
- L matmuls (M=32) run as two accumulation streams (even/odd f-tiles) in
  distinct 32-column PE groups via tile_position, sharing one PSUM bank.
- R matmuls run in fp8: x^2 is squared into fp8e4 (DVE tensor_mul for 3/5
  blocks, ACT Square activation for 2/5 — GpSimd is far too slow on HW),
  E'^2 is host-packed in fp8e4 scaled by 2^11 (dodging the fp8 subnormal
  floor; the epilogue multiplies by -2^-11). DoubleRow perf mode contracts
  two f-tiles per instruction at half the PE stream cost.
- The epilogue folds partition groups and computes L*L - R with 3 DVE ops.
"""

import math
import sys

if "/opt/trn_rl_repo" not in sys.path:
    sys.path.insert(0, "/opt/trn_rl_repo")

import numpy as np

N_ROWS = 20000
F = 10000
EK = 32
CORES = 8
FP = 128  # f-rows per tile (on SBUF partitions; 125 partitions cripples HW DMA)
FTILES = 80
F_PAD = FP * FTILES  # 10240 (f padded with zeros)
OCT = 16  # f-tiles per DMA block (double-octet: ~1.9 MB DMAs, fewer DVE ops)
NOCT = FTILES // OCT  # 5
MAXW = 512  # PSUM bank limit (f32 columns)
E2_SHIFT = 11  # e'^2 upload scale: 2^11 keeps values out of fp8 subnormals

_PROGRAM_CACHE: dict = {}


def _build_program(nch: int, w: int, repeat: int = 1, hw_loop: int = 1):
    """Per-core Bass program: nch chunks of w rows each (w <= 512, w % 16 == 0).

    repeat > 1 re-runs the whole compute that many times inside the program
    (idempotent; test-only, for overhead-free device timing via the r-slope).
    hw_loop > 1 wraps the compute in a hardware For_i loop instead (test-only;
    multiplies device work without growing the instruction count).
    """
    import concourse.mybir as mybir
    import concourse.tile as tile
    from concourse import bacc

    f32 = mybir.dt.float32
    bf16 = mybir.dt.bfloat16
    fp8 = mybir.dt.float8e4

    P = nch * w
    nc = bacc.Bacc("TRN2", target_bir_lowering=False, debug=False)
    # packed x: per partition p, flat index (c*FTILES + t)*w + j holds
    # x[row base_c + j, f = t*128 + p] (bf16, f >= 10000 zero-padded; tiles
    # must span all 128 partitions — 125-partition DMA is ~2.6x slower)
    xt = nc.dram_tensor("xt", [FP, FTILES * P], bf16, kind="ExternalInput")
    # packed E/sqrt(2): per partition p, flat t*EK + k = E'[t*128 + p, k]
    # (f >= 10000 zero-padded)
    emb = nc.dram_tensor("emb", [FP, FTILES * EK], bf16, kind="ExternalInput")
    # packed (E/sqrt(2))^2 * 2^E2_SHIFT in fp8e4, f-tile PAIRS interleaved for
    # DoubleRow: flat (j*2 + i)*EK + k = E2'[(2j+i)*128 + p, k]
    emb2 = nc.dram_tensor("emb2", [FP, FTILES * EK], fp8, kind="ExternalInput")
    outT = nc.dram_tensor("outT", [EK, P], f32, kind="ExternalOutput")

    with tile.TileContext(nc) as tc:
        with (
            tc.tile_pool(name="wpool", bufs=1) as wpool,
            tc.tile_pool(name="xpool", bufs=5) as xpool,
            tc.tile_pool(name="qpool", bufs=4) as qpool,
            tc.tile_pool(name="opool", bufs=2) as opool,
            tc.tile_pool(name="pspool", bufs=2, space="PSUM") as pspool,
        ):
            e_sb = wpool.tile([FP, FTILES, EK], bf16)
            nc.sync.dma_start(
                out=e_sb[:], in_=emb[:].rearrange("p (t k) -> p t k", t=FTILES)
            )
            e2_sb = wpool.tile([FP, FTILES // 2, 2, EK], fp8)
            nc.scalar.dma_start(
                out=e2_sb[:],
                in_=emb2[:].rearrange("p (j i k) -> p j i k", j=FTILES // 2, i=2),
            )

            def emit_chunk(c, tail_split=False):
                # bank A: L accumulates over even/odd f-tiles in partition
                # groups 0-31/32-63; bank B: R (DoubleRow needs dst base 0)
                psbA = pspool.tile([128, 512], f32, space="PSUM", name="psA")
                psbB = pspool.tile([128, 512], f32, space="PSUM", name="psB")
                ps = psbA[:, :w]
                psR = psbB[0:32, :w]
                # (t0, ntiles) DMA blocks; on the final chunk split the last
                # block into quarters so the end-of-kernel pipeline drain
                # (DMA -> square -> matmuls -> epilogue) is shorter
                blocks = [(o * OCT, OCT) for o in range(NOCT)]
                if tail_split:
                    t0 = blocks.pop()[0]
                    q = OCT // 4
                    blocks += [(t0 + i * q, q) for i in range(4)]
                for bi, (t0, nt) in enumerate(blocks):
                    x_sb = xpool.tile([FP, nt, w], bf16, name=f"x{nt}")
                    off = (c * FTILES + t0) * w
                    dma_eng = nc.sync if bi % 2 == 0 else nc.scalar
                    dma_eng.dma_start(
                        out=x_sb[:],
                        in_=xt[:, off : off + nt * w].rearrange(
                            "p (h j) -> p h j", h=nt
                        ),
                    )
                    xq_sb = qpool.tile([FP, nt, w], fp8, name=f"q{nt}")
                    # squares: DVE for 3/5 blocks, ACT (Square activation)
                    # for 2/5 — GpSimd is far too slow on real HW
                    if bi % 2 == 0:
                        nc.vector.tensor_mul(xq_sb[:], x_sb[:], x_sb[:])
                    else:
                        nc.scalar.activation(
                            out=xq_sb[:],
                            in_=x_sb[:],
                            func=mybir.ActivationFunctionType.Square,
                        )
                    for h in range(nt):
                        t = t0 + h
                        gL = 32 * (t & 1)
                        nc.tensor.matmul(
                            ps[gL : gL + 32, :],
                            e_sb[:, t, :],
                            x_sb[:, h, :],
                            start=(t < 2),
                            stop=(t >= FTILES - 2),
                            tile_position=(0, gL),
                            skip_group_check=True,
                        )
                    for i in range(nt // 2):
                        j = t0 // 2 + i  # f-tile pair index
                        nc.tensor.matmul(
                            psR,
                            e2_sb[:, j, :, :],
                            xq_sb[:, 2 * i : 2 * i + 2, :],
                            start=(j == 0),
                            stop=(j == FTILES // 2 - 1),
                            skip_group_check=True,
                            perf_mode=mybir.MatmulPerfMode.DoubleRow,
                        )
                # out = L^2 - R*2^-E2_SHIFT, L = g0 + g1, on DVE (GPSIMD
                # cannot access PSUM and is slow; DVE reads at most one PSUM
                # operand per instruction)
                lt = opool.tile([EK, w], f32, name="lt")
                nc.vector.tensor_copy(lt[:], ps[0:32, :])
                nc.vector.tensor_add(lt[:], lt[:], ps[32:64, :])
                osb = opool.tile([EK, w], f32, name="osb")
                nc.vector.tensor_mul(osb[:], lt[:], lt[:])
                nc.vector.scalar_tensor_tensor(
                    out=osb[:],
                    in0=psR,
                    scalar=-(2.0 ** -E2_SHIFT),
                    in1=osb[:],
                    op0=mybir.AluOpType.mult,
                    op1=mybir.AluOpType.add,
                )
                nc.sync.dma_start(out=outT[:, c * w : (c + 1) * w], in_=osb[:])

            if hw_loop > 1:
                with tc.For_i(0, hw_loop):
                    for c in range(nch):
                        emit_chunk(c)
            else:
                seq = [c for _ in range(repeat) for c in range(nch)]
                for k, c in enumerate(seq):
                    emit_chunk(c, tail_split=(k == len(seq) - 1))

    nc.compile()
    return nc


def _get_program(nch: int, w: int):
    key = (nch, w)
    if key not in _PROGRAM_CACHE:
        _PROGRAM_CACHE[key] = _build_program(nch, w)
    return _PROGRAM_CACHE[key]


def _np_dt(which: str):
    import concourse.mybir as mybir

    return mybir.dt.np(getattr(mybir.dt, which))


def _prepare_in_maps(input, emb_weight, train_idx):
    x = np.asarray(input, dtype=np.float32)
    e = np.asarray(emb_weight, dtype=np.float32)
    idx = np.asarray(train_idx).astype(np.int64)
    bf16 = _np_dt("bfloat16")
    fp8 = _np_dt("float8e4")

    rows = np.unique(idx)
    U = len(rows)
    if U == 0:
        return None, (0, 0), None  # no train rows: output is all zeros
    P0 = -(-U // CORES)
    nch = max(1, -(-P0 // MAXW))
    w = -(-(-(-P0 // nch)) // 16) * 16  # ceil(P0/nch) rounded up to x16
    P = nch * w
    # pad the row list with repeats of the last row (recomputed harmlessly)
    rows_pad = np.concatenate([rows, np.full(CORES * P - U, rows[-1], np.int64)])
    core_rows = rows_pad.reshape(CORES, P)

    ep = np.zeros((F_PAD, EK), dtype=np.float32)
    ep[:F] = e * np.float32(1.0 / math.sqrt(2.0))
    emb_bf = np.ascontiguousarray(
        ep.reshape(FTILES, FP, EK).transpose(1, 0, 2)
    ).reshape(FP, FTILES * EK).astype(bf16)
    e2 = (ep * ep) * np.float32(2.0 ** E2_SHIFT)
    emb2_f8 = np.ascontiguousarray(
        e2.reshape(FTILES, FP, EK).transpose(1, 0, 2)
    ).reshape(FP, FTILES * EK).astype(fp8)

    in_maps = []
    for c in range(CORES):
        xp = np.zeros((P, F_PAD), dtype=bf16)
        xp[:, :F] = x[core_rows[c]].astype(bf16)
        # [P, F_PAD] -> [p, c, t, j] so per-partition flat order is (c, t, j)
        a = xp.reshape(nch, w, FTILES, FP).transpose(3, 0, 2, 1)
        xt_host = np.ascontiguousarray(a).reshape(FP, FTILES * P)
        in_maps.append({"xt": xt_host, "emb": emb_bf, "emb2": emb2_f8})
    return in_maps, (nch, w), core_rows


def run_sharded(input, emb_weight, train_idx, trace: bool = False):
    """Run on 8 cores; returns (full_output, BassKernelResults)."""
    from concourse.bass_utils import run_bass_kernel_spmd

    in_maps, (nch, w), core_rows = _prepare_in_maps(input, emb_weight, train_idx)
    if in_maps is None:  # empty train_idx
        return np.zeros((N_ROWS, EK), dtype=np.float32), None
    nc = _get_program(nch, w)
    res = run_bass_kernel_spmd(
        nc, in_maps, core_ids=list(range(CORES)), trace=trace
    )
    out = np.zeros((N_ROWS, EK), dtype=np.float32)
    for c in range(CORES):
        out[core_rows[c]] = res.results[c]["outT"].T
    return out, res


def kernel(input, emb_weight, train_idx):
    out, _ = run_sharded(input, emb_weight, train_idx)
    return out


# revision 31
# speedup vs baseline: 1.3750x; 1.3750x over previous
"""FM bi-interaction (embedding_lookup) Trainium2 kernel.

out[n, k] = 0.5 * ((x @ E)^2 - (x*x) @ (E*E))[n, k] * mask[n]
mask[n] = 1 if n in train_idx else 0

Strategy (data-parallel over rows, 8 NeuronCores):
- Only rows present in train_idx have nonzero output (~11k of 20k). The host
  gathers the unique train rows, splits them evenly across the 8 cores, and
  scatters the per-row results back into a zero output — no on-device mask.
- x is uploaded in bf16 (the 2e-2 rel-err gate leaves ~40x headroom), halving
  HBM traffic; E is pre-scaled by 1/sqrt(2) on the host so the 0.5 factor
  folds into the matmuls (out = L^2 - R with L = x@E', R = x^2@E'^2).
- Host packs x into the exact SBUF tile layout ([128 f-partitions, 16
  f-tiles, w rows] per block, f padded 10000->10240 — tiles must span all 128
  partitions or DMA throughput collapses), so every x DMA is one ~1.9 MB
  transfer with fully contiguous per-partition lines. DMAs alternate between
  the SP and ACT HWDGE rings to overlap.
- L matmuls (M=32) run as two accumulation streams (even/odd f-tiles) in
  distinct 32-column PE groups via tile_position, sharing one PSUM bank.
- R matmuls run in fp8: x^2 is squared into fp8e4 (DVE tensor_mul for 3/5
  blocks, ACT Square activation for 2/5 — GpSimd is far too slow on HW),
  E'^2 is host-packed in fp8e4 scaled by 2^11 (dodging the fp8 subnormal
  floor; the epilogue multiplies by -2^-11). DoubleRow perf mode contracts
  two f-tiles per instruction at half the PE stream cost.
- The epilogue folds partition groups and computes L*L - R with 4 DVE ops;
  the final chunk's last DMA block is split into quarters to shorten the
  end-of-kernel pipeline drain.
"""

import math
import sys

if "/opt/trn_rl_repo" not in sys.path:
    sys.path.insert(0, "/opt/trn_rl_repo")

import numpy as np

N_ROWS = 20000
F = 10000
EK = 32
CORES = 8
FP = 128  # f-rows per tile (on SBUF partitions; 125 partitions cripples HW DMA)
FTILES = 80
F_PAD = FP * FTILES  # 10240 (f padded with zeros)
OCT = 16  # f-tiles per DMA block (double-octet: ~1.9 MB DMAs, fewer DVE ops)
NOCT = FTILES // OCT  # 5
MAXW = 512  # PSUM bank limit (f32 columns)
E2_SHIFT = 11  # e'^2 upload scale: 2^11 keeps values out of fp8 subnormals

_PROGRAM_CACHE: dict = {}


def _build_program(nch: int, w: int, repeat: int = 1, hw_loop: int = 1):
    """Per-core Bass program: nch chunks of w rows each (w <= 512, w % 16 == 0).

    repeat > 1 re-runs the whole compute that many times inside the program
    (idempotent; test-only, for overhead-free device timing via the r-slope).
    hw_loop > 1 wraps the compute in a hardware For_i loop instead (test-only;
    multiplies device work without growing the instruction count).
    """
    import concourse.mybir as mybir
    import concourse.tile as tile
    from concourse import bacc

    f32 = mybir.dt.float32
    bf16 = mybir.dt.bfloat16
    fp8 = mybir.dt.float8e4

    P = nch * w
    nc = bacc.Bacc("TRN2", target_bir_lowering=False, debug=False)
    # packed x: per partition p, flat index (c*FTILES + t)*w + j holds
    # x[row base_c + j, f = t*128 + p] (bf16, f >= 10000 zero-padded; tiles
    # must span all 128 partitions — 125-partition DMA is ~2.6x slower)
    xt = nc.dram_tensor("xt", [FP, FTILES * P], bf16, kind="ExternalInput")
    # packed E/sqrt(2): per partition p, flat t*EK + k = E'[t*128 + p, k]
    # (f >= 10000 zero-padded)
    emb = nc.dram_tensor("emb", [FP, FTILES * EK], bf16, kind="ExternalInput")
    # packed (E/sqrt(2))^2 * 2^E2_SHIFT in fp8e4, f-tile PAIRS interleaved for
    # DoubleRow: flat (j*2 + i)*EK + k = E2'[(2j+i)*128 + p, k]
    emb2 = nc.dram_tensor("emb2", [FP, FTILES * EK], fp8, kind="ExternalInput")
    outT = nc.dram_tensor("outT", [EK, P], f32, kind="ExternalOutput")

    with tile.TileContext(nc) as tc:
        with (
            tc.tile_pool(name="wpool", bufs=1) as wpool,
            tc.tile_pool(name="xpool", bufs=5) as xpool,
            tc.tile_pool(name="qpool", bufs=4) as qpool,
            tc.tile_pool(name="opool", bufs=2) as opool,
            tc.tile_pool(name="pspool", bufs=2, space="PSUM") as pspool,
        ):
            e_sb = wpool.tile([FP, FTILES, EK], bf16)
            nc.sync.dma_start(
                out=e_sb[:], in_=emb[:].rearrange("p (t k) -> p t k", t=FTILES)
            )
            e2_sb = wpool.tile([FP, FTILES // 2, 2, EK], fp8)
            nc.scalar.dma_start(
                out=e2_sb[:],
                in_=emb2[:].rearrange("p (j i k) -> p j i k", j=FTILES // 2, i=2),
            )

            def emit_chunk(c, tail_split=False):
                # bank A: L accumulates over even/odd f-tiles in partition
                # groups 0-31/32-63; bank B: R (DoubleRow needs dst base 0)
                psbA = pspool.tile([128, 512], f32, space="PSUM", name="psA")
                psbB = pspool.tile([128, 512], f32, space="PSUM", name="psB")
                ps = psbA[:, :w]
                psR = psbB[0:32, :w]
                # (t0, ntiles) DMA blocks; on the final chunk split the last
                # block into quarters so the end-of-kernel pipeline drain
                # (DMA -> square -> matmuls -> epilogue) is shorter
                blocks = [(o * OCT, OCT) for o in range(NOCT)]
                if tail_split:
                    t0 = blocks.pop()[0]
                    q = OCT // 4
                    blocks += [(t0 + i * q, q) for i in range(4)]
                for bi, (t0, nt) in enumerate(blocks):
                    x_sb = xpool.tile([FP, nt, w], bf16, name=f"x{nt}")
                    off = (c * FTILES + t0) * w
                    # x DMAs rotate over three rings: SP + ACT (HWDGE) and
                    # GpSimd (SWDGE) — lifts aggregate HBM pull a few percent
                    dma_eng = (nc.sync, nc.scalar, nc.gpsimd, nc.sync, nc.scalar)[
                        bi % 5
                    ]
                    dma_eng.dma_start(
                        out=x_sb[:],
                        in_=xt[:, off : off + nt * w].rearrange(
                            "p (h j) -> p h j", h=nt
                        ),
                    )
                    xq_sb = qpool.tile([FP, nt, w], fp8, name=f"q{nt}")
                    # squares: DVE for 3/5 blocks, ACT (Square activation)
                    # for 2/5 — GpSimd is far too slow on real HW
                    if bi % 2 == 0:
                        nc.vector.tensor_mul(xq_sb[:], x_sb[:], x_sb[:])
                    else:
                        nc.scalar.activation(
                            out=xq_sb[:],
                            in_=x_sb[:],
                            func=mybir.ActivationFunctionType.Square,
                        )
                    for h in range(nt):
                        t = t0 + h
                        gL = 32 * (t & 1)
                        nc.tensor.matmul(
                            ps[gL : gL + 32, :],
                            e_sb[:, t, :],
                            x_sb[:, h, :],
                            start=(t < 2),
                            stop=(t >= FTILES - 2),
                            tile_position=(0, gL),
                            skip_group_check=True,
                        )
                    for i in range(nt // 2):
                        j = t0 // 2 + i  # f-tile pair index
                        nc.tensor.matmul(
                            psR,
                            e2_sb[:, j, :, :],
                            xq_sb[:, 2 * i : 2 * i + 2, :],
                            start=(j == 0),
                            stop=(j == FTILES // 2 - 1),
                            skip_group_check=True,
                            perf_mode=mybir.MatmulPerfMode.DoubleRow,
                        )
                # out = L^2 - R*2^-E2_SHIFT, L = g0 + g1, on DVE (GPSIMD
                # cannot access PSUM and is slow; DVE reads at most one PSUM
                # operand per instruction)
                lt = opool.tile([EK, w], f32, name="lt")
                nc.vector.tensor_copy(lt[:], ps[0:32, :])
                nc.vector.tensor_add(lt[:], lt[:], ps[32:64, :])
                osb = opool.tile([EK, w], f32, name="osb")
                nc.vector.tensor_mul(osb[:], lt[:], lt[:])
                nc.vector.scalar_tensor_tensor(
                    out=osb[:],
                    in0=psR,
                    scalar=-(2.0 ** -E2_SHIFT),
                    in1=osb[:],
                    op0=mybir.AluOpType.mult,
                    op1=mybir.AluOpType.add,
                )
                # output writes go on the ACT ring, keeping SP free for x
                nc.scalar.dma_start(out=outT[:, c * w : (c + 1) * w], in_=osb[:])

            if hw_loop > 1:
                with tc.For_i(0, hw_loop):
                    for c in range(nch):
                        emit_chunk(c)
            else:
                seq = [c for _ in range(repeat) for c in range(nch)]
                for k, c in enumerate(seq):
                    emit_chunk(c, tail_split=(k == len(seq) - 1))

    nc.compile()
    return nc


def _get_program(nch: int, w: int):
    key = (nch, w)
    if key not in _PROGRAM_CACHE:
        _PROGRAM_CACHE[key] = _build_program(nch, w)
    return _PROGRAM_CACHE[key]


def _np_dt(which: str):
    import concourse.mybir as mybir

    return mybir.dt.np(getattr(mybir.dt, which))


def _prepare_in_maps(input, emb_weight, train_idx):
    x = np.asarray(input, dtype=np.float32)
    e = np.asarray(emb_weight, dtype=np.float32)
    idx = np.asarray(train_idx).astype(np.int64)
    bf16 = _np_dt("bfloat16")
    fp8 = _np_dt("float8e4")

    rows = np.unique(idx)
    U = len(rows)
    if U == 0:
        return None, (0, 0), None  # no train rows: output is all zeros
    P0 = -(-U // CORES)
    nch = max(1, -(-P0 // MAXW))
    w = -(-(-(-P0 // nch)) // 16) * 16  # ceil(P0/nch) rounded up to x16
    P = nch * w
    # pad the row list with repeats of the last row (recomputed harmlessly)
    rows_pad = np.concatenate([rows, np.full(CORES * P - U, rows[-1], np.int64)])
    core_rows = rows_pad.reshape(CORES, P)

    ep = np.zeros((F_PAD, EK), dtype=np.float32)
    ep[:F] = e * np.float32(1.0 / math.sqrt(2.0))
    emb_bf = np.ascontiguousarray(
        ep.reshape(FTILES, FP, EK).transpose(1, 0, 2)
    ).reshape(FP, FTILES * EK).astype(bf16)
    e2 = (ep * ep) * np.float32(2.0 ** E2_SHIFT)
    emb2_f8 = np.ascontiguousarray(
        e2.reshape(FTILES, FP, EK).transpose(1, 0, 2)
    ).reshape(FP, FTILES * EK).astype(fp8)

    in_maps = []
    for c in range(CORES):
        xp = np.zeros((P, F_PAD), dtype=bf16)
        xp[:, :F] = x[core_rows[c]].astype(bf16)
        # [P, F_PAD] -> [p, c, t, j] so per-partition flat order is (c, t, j)
        a = xp.reshape(nch, w, FTILES, FP).transpose(3, 0, 2, 1)
        xt_host = np.ascontiguousarray(a).reshape(FP, FTILES * P)
        in_maps.append({"xt": xt_host, "emb": emb_bf, "emb2": emb2_f8})
    return in_maps, (nch, w), core_rows


def run_sharded(input, emb_weight, train_idx, trace: bool = False):
    """Run on 8 cores; returns (full_output, BassKernelResults)."""
    from concourse.bass_utils import run_bass_kernel_spmd

    in_maps, (nch, w), core_rows = _prepare_in_maps(input, emb_weight, train_idx)
    if in_maps is None:  # empty train_idx
        return np.zeros((N_ROWS, EK), dtype=np.float32), None
    nc = _get_program(nch, w)
    res = run_bass_kernel_spmd(
        nc, in_maps, core_ids=list(range(CORES)), trace=trace
    )
    out = np.zeros((N_ROWS, EK), dtype=np.float32)
    for c in range(CORES):
        out[core_rows[c]] = res.results[c]["outT"].T
    return out, res


def kernel(input, emb_weight, train_idx):
    out, _ = run_sharded(input, emb_weight, train_idx)
    return out
